# revision 2
# baseline (speedup 1.0000x reference)
"""Trainium2 Bass kernel for nn_Camada_33612414059004.

Computes, for x:[B,N,D,S], M:[N,N], w_syn:[N,D,S], b_dend:[N,D],
w_dend:[N,D], b_soma:[N]:

    xm    = einsum('bids,oi->bods', x, M)
    dend  = tanh(einsum('bnds,nds->bnd', xm, w_syn) + b_dend)
    soma  = einsum('bnd,nd->bn', dend, w_dend) + b_soma
    out   = sigmoid(soma)                                  # [B, N]

Sharding: data-parallel over batch across 8 NeuronCores (B=64 -> 8/core),
zero cross-core communication.

Per core the dominant work is the connectivity matmul M[o,i] @ x[i,(b,d,s)]
([1024x1024]x[1024x1024], 2.15 GFLOP).  It runs in fp8 E4M3 with DoubleRow
perf mode (two 128-row contraction planes per instruction, 2x PE rate,
fp32 PSUM accumulate): 8 o-tiles x 2 halves x 4 chunk-pairs = 64 matmuls,
~13.7us of PE time at 2.4 GHz.  End-to-end numeric error vs the fp32
reference is ~0.5% (validated on CPU), well inside the 2e-2 gate: M is 0/1
(exact in fp8) and the tanh/sigmoid stages compress the fp8 input noise.

Inputs stream as 4 chunk-pairs per operand: x chunks on the Sync HWDGE
ring, M^T chunks on the Scalar HWDGE ring (parallel), per-neuron params
via the GpSimd SWDGE path behind the first x chunk.  fp8 halves the DMA
volume to ~2.3 MB/core.

Postprocess per o-tile is spread so no engine exceeds the PE's pace:
Vector multiplies PSUM by w_syn (bf16 products) and does the small soma
stage, GpSimd does the s-reduction tree + bias, Scalar does tanh /
sigmoid(+b_soma).  O-tiles run in two waves of 4 (PSUM holds 4 full
[128,1024] f32 accumulators): chunk-0 matmuls for the wave ride the DMA
stream, then each tile finishes its remaining chunks and its postprocess
chain pipelines against the next tile's matmuls.  PE pre-warm dummy
matmuls lift the HAM clock gate (1.2->2.4 GHz) during the DMA wait.
"""

import numpy as np
import ml_dtypes
from contextlib import ExitStack

import concourse.bass as bass
import concourse.mybir as mybir
import concourse.tile as tile

B, N, D, S = 64, 1024, 8, 16
NCORES = 8
BC = B // NCORES          # batches per core = 8
DS = D * S                # 128
P = 128                   # SBUF partitions
C = 4                     # contraction chunk-pairs (256 input rows each)
OT = N // P               # 8 output-neuron tiles
BD = BC * D               # 64
FH = 512                  # one fp32 PSUM bank of moving free dim
PMW1 = OT * D             # w_dend offset in pm
PMB1 = 2 * OT * D         # b_soma offset in pm
PMCOLS = 2 * OT * D + OT  # 136

F32 = mybir.dt.float32
BF16 = mybir.dt.bfloat16
F8 = mybir.dt.float8e4
DR = mybir.MatmulPerfMode.DoubleRow

_NC_CACHE = {}


def legalize_waits(nc, max_attached=1):
    """Split multi-semaphore waits onto preceding same-engine NOPs.

    The walrus build in this environment accepts at most one sync-wait
    command per instruction (setupSyncWait: "Too many sync wait commands"),
    but Tile attaches one wait per out-of-date engine clock.  An engine is
    in-order, so hoisting the extra waits onto NOPs immediately before the
    instruction is semantics-preserving.
    """
    nid = 0
    for f in nc.m.functions:
        for blk in f.blocks:
            new = []
            changed = False
            for inst in blk.instructions:
                si = inst.sync_info
                if si is not None and si.on_wait and len(si.on_wait) > max_attached:
                    waits = list(si.on_wait)
                    for w in waits[:-max_attached]:
                        nid += 1
                        nop = mybir.InstNoOp(name=f"WSPLIT-{nid}", ins=[], outs=[])
                        nop.engine = inst.engine
                        nop.sync_info = mybir.SyncInfo(on_wait=[w], on_update=[])
                        new.append(nop)
                    inst.sync_info = mybir.SyncInfo(
                        on_wait=waits[-max_attached:], on_update=list(si.on_update)
                    )
                    changed = True
                new.append(inst)
            if changed:
                blk.instructions = new
    return nc


def build_nc(legalize=True):
    """Build the single-core Bass program (SPMD: same program on all cores)."""
    nc = bass.Bass()
    mt = nc.declare_dram_parameter("mt", [C * P, 2 * N], F8, isOutput=False)
    xc = nc.declare_dram_parameter("xc", [C * P, 2 * BC * DS], F8, isOutput=False)
    wsyn = nc.declare_dram_parameter("wsyn", [P, OT * DS], BF16, isOutput=False)
    pm = nc.declare_dram_parameter("pm", [P, PMCOLS], F32, isOutput=False)
    out = nc.declare_dram_parameter("out", [P, OT * BC], F32, isOutput=True)

    AF = mybir.ActivationFunctionType
    AX = mybir.AxisListType
    OP = mybir.AluOpType

    with tile.TileContext(nc) as tc, ExitStack() as ctx:
        wpool = ctx.enter_context(tc.tile_pool(name="weights", bufs=1))
        xpool = ctx.enter_context(tc.tile_pool(name="xin", bufs=1))
        pspool = ctx.enter_context(tc.tile_pool(name="ps", bufs=4, space="PSUM"))
        prpool = ctx.enter_context(tc.tile_pool(name="prp", bufs=3))
        smpool = ctx.enter_context(tc.tile_pool(name="smp", bufs=3))

        # --- PE pre-warm scratch: zeroed fp8 tile, matmuls issued below.
        # The HAM clock gate needs ~3.4us of sustained PE activity to lift
        # the PE from 1.2 to 2.4 GHz; warming during the DMA wait means the
        # real matmuls run at full rate (or close) from the start. ---
        warm_sb = wpool.tile([P, FH], F8, tag="warm", name="warm_sb")
        nc.gpsimd.memset(warm_sb[:], 0.0)

        # --- input DMAs: x chunk-pairs on Sync, mt chunk-pairs on Scalar
        # (parallel HWDGE rings); whole [128, 2KB-row] chunks. ---
        x_tiles, mt_tiles = [], []
        x0_dma = None
        for c in range(C):
            xt = xpool.tile([P, 2 * BC * DS], F8, tag=f"x{c}", name=f"x{c}")
            xdma = nc.sync.dma_start(xt[:], xc[c * P:(c + 1) * P, :])
            if c == 0:
                x0_dma = xdma
            x_tiles.append(xt)
        for c in range(C):
            mtk = xpool.tile([P, 2 * N], F8, tag=f"m{c}", name=f"m{c}")
            nc.scalar.dma_start(mtk[:], mt[c * P:(c + 1) * P, :])
            mt_tiles.append(mtk)

        # Per-neuron parameters on the GpSimd SWDGE path, delayed behind the
        # first (matmul-critical) x chunk.
        wsyn_sb = wpool.tile([P, OT * DS], BF16, tag="wsyn", name="wsyn_sb")
        pm_sb = wpool.tile([P, PMCOLS], F32, tag="pm", name="pm_sb")
        from bass_rust import add_dep_helper
        wdma = nc.gpsimd.dma_start(wsyn_sb[:], wsyn[:, :])
        add_dep_helper(wdma.ins, x0_dma.ins, sync=True,
                       reason="params after critical first chunk")
        nc.gpsimd.dma_start(pm_sb[:], pm[:, :])

        out_sb = wpool.tile([P, OT * BC], F32, tag="out", name="out_sb")

        # Dummy activation to pull the ACT table load (~2.7us) into the DMA
        # wait instead of the first real tanh.
        scratch = smpool.tile([P, 1], F32, tag="scr", name="scratch")
        nc.scalar.activation(scratch[:], warm_sb[:, 0:1], AF.Tanh)

        # --- PE warm-up: 16 small DoubleRow matmuls on the zero tile. ---
        warm_ps = pspool.tile([P, 2 * FH], F32, tag="ps", name="warm_ps")
        wv = warm_sb[:].rearrange("p (j f) -> p j f", j=2)
        for _ in range(16):
            nc.tensor.matmul(
                warm_ps[:, 0:2 * P], lhsT=wv[:, :, 0:P], rhs=wv,
                start=True, stop=True, perf_mode=DR,
            )

        def mm(pst, t, c):
            mtv = mt_tiles[c][:].rearrange("p (j o) -> p j o", j=2)
            xv = x_tiles[c][:].rearrange("p (j f) -> p j f", j=2)
            for h in range(2):
                nc.tensor.matmul(
                    pst[:, h * FH:(h + 1) * FH],
                    lhsT=mtv[:, :, t * P:(t + 1) * P],
                    rhs=xv[:, :, h * FH:(h + 1) * FH],
                    start=(c == 0), stop=(c == C - 1), perf_mode=DR,
                )

        def postprocess(t, pst):
            # prod[o, b, (d,s)] = xm * w_syn (broadcast over b), read
            # straight from PSUM, bf16 products.
            prod = prpool.tile([P, BC * DS], BF16, tag="prod", name=f"prod{t}")
            nc.vector.tensor_mul(
                prod[:].rearrange("p (b f) -> p b f", b=BC),
                pst[:].rearrange("p (b f) -> p b f", b=BC),
                wsyn_sb[:, t * DS:(t + 1) * DS].unsqueeze(1)
                .broadcast_to([P, BC, DS]),
            )
            # s-reduce as a GpSimd pairwise tree (frees the in-order DVE for
            # the next tile's PSUM mult), + bias.
            pv = prod[:].rearrange("p (bd s) -> p bd s", s=S)
            gr1 = smpool.tile([P, BD * 8], F32, tag="gr1", name=f"gr1{t}")
            nc.gpsimd.tensor_add(
                gr1[:].rearrange("p (bd s) -> p bd s", s=8),
                pv[:, :, 0:8], pv[:, :, 8:16],
            )
            g1v = gr1[:].rearrange("p (bd s) -> p bd s", s=8)
            gr2 = smpool.tile([P, BD * 4], F32, tag="gr2", name=f"gr2{t}")
            nc.gpsimd.tensor_add(
                gr2[:].rearrange("p (bd s) -> p bd s", s=4),
                g1v[:, :, 0:4], g1v[:, :, 4:8],
            )
            g2v = gr2[:].rearrange("p (bd s) -> p bd s", s=4)
            gr3 = smpool.tile([P, BD * 2], F32, tag="gr3", name=f"gr3{t}")
            nc.gpsimd.tensor_add(
                gr3[:].rearrange("p (bd s) -> p bd s", s=2),
                g2v[:, :, 0:2], g2v[:, :, 2:4],
            )
            g3v = gr3[:].rearrange("p (bd s) -> p bd s", s=2)
            dp = smpool.tile([P, BD], F32, tag="dp", name=f"dp{t}")
            nc.gpsimd.tensor_add(
                dp[:].unsqueeze(2), g3v[:, :, 0:1], g3v[:, :, 1:2],
            )
            nc.gpsimd.tensor_add(
                dp[:].rearrange("p (b d) -> p b d", d=D),
                dp[:].rearrange("p (b d) -> p b d", d=D),
                pm_sb[:, t * D:(t + 1) * D].unsqueeze(1)
                .broadcast_to([P, BC, D]),
            )
            dend = smpool.tile([P, BD], F32, tag="dend", name=f"dend{t}")
            nc.scalar.activation(dend[:], dp[:], AF.Tanh)
            # soma: * w_dend, reduce over d, sigmoid(+b_soma) -- small, DVE.
            sp = smpool.tile([P, BD], F32, tag="sp", name=f"sp{t}")
            nc.vector.tensor_mul(
                sp[:].rearrange("p (b d) -> p b d", d=D),
                dend[:].rearrange("p (b d) -> p b d", d=D),
                pm_sb[:, PMW1 + t * D:PMW1 + (t + 1) * D].unsqueeze(1)
                .broadcast_to([P, BC, D]),
            )
            soma = smpool.tile([P, BC], F32, tag="soma", name=f"soma{t}")
            nc.vector.tensor_reduce(
                soma[:], sp[:].rearrange("p (b d) -> p b d", d=D),
                axis=AX.X, op=OP.add,
            )
            nc.scalar.activation(
                out_sb[:, t * BC:(t + 1) * BC], soma[:], AF.Sigmoid,
                bias=pm_sb[:, PMB1 + t:PMB1 + t + 1],
            )

        # Two waves of 4 o-tiles (PSUM fits 4 [128,1024] f32 accumulators).
        # Chunk-0 matmuls for the whole wave ride the DMA stream; then each
        # tile finishes chunks 1..3 and its postprocess chain pipelines
        # against the next tile's matmuls.
        for w in range(2):
            tiles = range(w * 4, w * 4 + 4)
            pst = {}
            for t in tiles:
                pst[t] = pspool.tile([P, 2 * FH], F32, tag="ps", name=f"ps{t}")
                mm(pst[t], t, 0)
            for t in tiles:
                for c in range(1, C):
                    mm(pst[t], t, c)
                postprocess(t, pst[t])

        nc.sync.dma_start(out[:, :], out_sb[:])

    if legalize:
        legalize_waits(nc)
    return nc


def get_nc():
    if "nc" not in _NC_CACHE:
        _NC_CACHE["nc"] = build_nc()
    return _NC_CACHE["nc"]


def _pair_chunks(a):
    """[N, F] -> [C*P, 2*F] with row c*128+p holding planes (j=0, j=1) of
    chunk-pair c: out[c*128+p, j*F+f] = a[256c + 128j + p, f]."""
    Fdim = a.shape[1]
    return np.ascontiguousarray(
        a.reshape(C, 2, P, Fdim).transpose(0, 2, 1, 3).reshape(C * P, 2 * Fdim)
    )


def prepare_in_maps(x, matriz_conexao, w_syn, b_dend, w_dend, b_soma):
    f8 = ml_dtypes.float8_e4m3
    x = np.asarray(x, dtype=np.float32)
    mt_np = _pair_chunks(
        np.ascontiguousarray(np.asarray(matriz_conexao, np.float32).T)
    ).astype(f8)
    wsyn_np = np.ascontiguousarray(
        np.asarray(w_syn, np.float32).reshape(OT, P, DS)
        .transpose(1, 0, 2).reshape(P, OT * DS)
    ).astype(ml_dtypes.bfloat16)
    bd = np.asarray(b_dend, np.float32).reshape(OT, P, D).transpose(1, 0, 2).reshape(P, OT * D)
    wd = np.asarray(w_dend, np.float32).reshape(OT, P, D).transpose(1, 0, 2).reshape(P, OT * D)
    bs = np.asarray(b_soma, np.float32).reshape(OT, P).T
    pm_np = np.ascontiguousarray(np.concatenate([bd, wd, bs], axis=1))
    xt = x.transpose(1, 0, 2, 3).reshape(N, B, DS)
    in_maps = []
    for c in range(NCORES):
        xc_np = _pair_chunks(
            np.ascontiguousarray(
                xt[:, c * BC:(c + 1) * BC, :].reshape(N, BC * DS))
        ).astype(f8)
        in_maps.append({"mt": mt_np, "xc": xc_np, "wsyn": wsyn_np, "pm": pm_np})
    return in_maps


def assemble_output(results):
    outs = []
    for c in range(NCORES):
        oc = np.asarray(results[c]["out"])          # [P, OT*BC] = (oi, (t, b))
        outs.append(oc.reshape(P, OT, BC).transpose(2, 1, 0).reshape(BC, N))
    return np.ascontiguousarray(np.concatenate(outs, axis=0).astype(np.float32))


def kernel(x, matriz_conexao, w_syn, b_dend, w_dend, b_soma):
    from concourse.bass_utils import run_bass_kernel_spmd
    in_maps = prepare_in_maps(x, matriz_conexao, w_syn, b_dend, w_dend, b_soma)
    nc = get_nc()
    res = run_bass_kernel_spmd(nc, in_maps, list(range(NCORES)))
    return assemble_output(res.results)


# revision 7
# speedup vs baseline: 1.1899x; 1.1899x over previous
"""Trainium2 Bass kernel for nn_Camada_33612414059004.

Computes, for x:[B,N,D,S], M:[N,N], w_syn:[N,D,S], b_dend:[N,D],
w_dend:[N,D], b_soma:[N]:

    xm    = einsum('bids,oi->bods', x, M)
    dend  = tanh(einsum('bnds,nds->bnd', xm, w_syn) + b_dend)
    soma  = einsum('bnd,nd->bn', dend, w_dend) + b_soma
    out   = sigmoid(soma)                                  # [B, N]

Sharding: data-parallel over batch across 8 NeuronCores (B=64 -> 8/core),
zero cross-core communication.

Per core the dominant work is the connectivity matmul M[o,i] @ x[i,(b,d,s)]
([1024x1024]x[1024x1024], 2.15 GFLOP).  It runs in fp8 E4M3 with DoubleRow
perf mode (two 128-row contraction planes per instruction, 2x PE rate,
fp32 PSUM accumulate): 8 o-tiles x 2 halves x 4 chunk-pairs = 64 matmuls,
~13.7us of PE time at 2.4 GHz.  End-to-end numeric error vs the fp32
reference is ~0.5% (validated on CPU), well inside the 2e-2 gate: M is 0/1
(exact in fp8) and the tanh/sigmoid stages compress the fp8 input noise.

Inputs stream as 4 chunk-pairs per operand: x chunks on the Sync HWDGE
ring, M^T chunks on the Scalar HWDGE ring (parallel), per-neuron params
via the GpSimd SWDGE path behind the first x chunk.  fp8 halves the DMA
volume to ~2.3 MB/core.

Postprocess per o-tile is spread so no engine exceeds the PE's pace:
Vector multiplies PSUM by w_syn (bf16 products) and does the small soma
stage, GpSimd does the s-reduction tree + bias, Scalar does tanh /
sigmoid(+b_soma).  O-tiles run in two waves of 4 (PSUM holds 4 full
[128,1024] f32 accumulators): chunk-0 matmuls for the wave ride the DMA
stream, then each tile finishes its remaining chunks and its postprocess
chain pipelines against the next tile's matmuls.  PE pre-warm dummy
matmuls lift the HAM clock gate (1.2->2.4 GHz) during the DMA wait.
"""

import numpy as np
import ml_dtypes
from contextlib import ExitStack

import concourse.bass as bass
import concourse.mybir as mybir
import concourse.tile as tile

B, N, D, S = 64, 1024, 8, 16
NCORES = 8
BC = B // NCORES          # batches per core = 8
DS = D * S                # 128
P = 128                   # SBUF partitions
C = 4                     # contraction chunk-pairs (256 input rows each)
OT = N // P               # 8 output-neuron tiles
BD = BC * D               # 64
FH = 512                  # one fp32 PSUM bank of moving free dim
PMW1 = OT * D             # w_dend offset in pm
PMB1 = 2 * OT * D         # b_soma offset in pm
PMCOLS = 2 * OT * D + OT  # 136

F32 = mybir.dt.float32
BF16 = mybir.dt.bfloat16
F8 = mybir.dt.float8e4
DR = mybir.MatmulPerfMode.DoubleRow

_NC_CACHE = {}


def legalize_waits(nc, max_attached=1):
    """Split multi-semaphore waits onto preceding same-engine NOPs.

    The walrus build in this environment accepts at most one sync-wait
    command per instruction (setupSyncWait: "Too many sync wait commands"),
    but Tile attaches one wait per out-of-date engine clock.  An engine is
    in-order, so hoisting the extra waits onto NOPs immediately before the
    instruction is semantics-preserving.
    """
    nid = 0
    for f in nc.m.functions:
        for blk in f.blocks:
            new = []
            changed = False
            for inst in blk.instructions:
                si = inst.sync_info
                if si is not None and si.on_wait and len(si.on_wait) > max_attached:
                    waits = list(si.on_wait)
                    for w in waits[:-max_attached]:
                        nid += 1
                        nop = mybir.InstNoOp(name=f"WSPLIT-{nid}", ins=[], outs=[])
                        nop.engine = inst.engine
                        nop.sync_info = mybir.SyncInfo(on_wait=[w], on_update=[])
                        new.append(nop)
                    inst.sync_info = mybir.SyncInfo(
                        on_wait=waits[-max_attached:], on_update=list(si.on_update)
                    )
                    changed = True
                new.append(inst)
            if changed:
                blk.instructions = new
    return nc


def build_nc(legalize=True):
    """Build the single-core Bass program (SPMD: same program on all cores)."""
    nc = bass.Bass()
    mt = nc.declare_dram_parameter("mt", [C * P, 2 * N], F8, isOutput=False)
    xc = nc.declare_dram_parameter("xc", [C * P, 2 * BC * DS], F8, isOutput=False)
    wsyn = nc.declare_dram_parameter("wsyn", [P, OT * DS], BF16, isOutput=False)
    pm = nc.declare_dram_parameter("pm", [P, PMCOLS], F32, isOutput=False)
    out = nc.declare_dram_parameter("out", [P, OT * BC], F32, isOutput=True)

    AF = mybir.ActivationFunctionType
    AX = mybir.AxisListType
    OP = mybir.AluOpType

    with tile.TileContext(nc) as tc, ExitStack() as ctx:
        wpool = ctx.enter_context(tc.tile_pool(name="weights", bufs=1))
        xpool = ctx.enter_context(tc.tile_pool(name="xin", bufs=1))
        pspool = ctx.enter_context(tc.tile_pool(name="ps", bufs=4, space="PSUM"))
        prpool = ctx.enter_context(tc.tile_pool(name="prp", bufs=3))
        smpool = ctx.enter_context(tc.tile_pool(name="smp", bufs=3))

        # --- PE pre-warm scratch: zeroed fp8 tile, matmuls issued below.
        # The HAM clock gate needs ~3.4us of sustained PE activity to lift
        # the PE from 1.2 to 2.4 GHz; warming during the DMA wait means the
        # real matmuls run at full rate (or close) from the start. ---
        warm_sb = wpool.tile([P, FH], F8, tag="warm", name="warm_sb")
        nc.gpsimd.memset(warm_sb[:], 0.0)

        # --- input DMAs: x chunk-pairs on Sync, mt chunk-pairs on Scalar
        # (parallel HWDGE rings); whole [128, 2KB-row] chunks. ---
        x_tiles, mt_tiles = [], []
        mt_dmas = []
        for c in range(C):
            xt = xpool.tile([P, 2 * BC * DS], F8, tag=f"x{c}", name=f"x{c}")
            nc.sync.dma_start(xt[:], xc[c * P:(c + 1) * P, :])
            x_tiles.append(xt)
        for c in range(C):
            mtk = xpool.tile([P, 2 * N], F8, tag=f"m{c}", name=f"m{c}")
            mt_dmas.append(nc.scalar.dma_start(mtk[:], mt[c * P:(c + 1) * P, :]))
            mt_tiles.append(mtk)

        # Per-neuron parameters on the GpSimd SWDGE path, delayed behind the
        # second mt chunk so they don't contend with the matmul-critical
        # chunk-0/1 streams (wsyn is first needed at tile 0's postprocess,
        # well after chunk 3 lands).
        wsyn_sb = wpool.tile([P, OT * DS], BF16, tag="wsyn", name="wsyn_sb")
        pm_sb = wpool.tile([P, PMCOLS], F32, tag="pm", name="pm_sb")
        from bass_rust import add_dep_helper
        wdma = nc.gpsimd.dma_start(wsyn_sb[:], wsyn[:, :])
        add_dep_helper(wdma.ins, mt_dmas[1].ins, sync=True,
                       reason="params after critical early chunks")
        nc.gpsimd.dma_start(pm_sb[:], pm[:, :])

        out_sb = wpool.tile([P, OT * BC], F32, tag="out", name="out_sb")

        # Dummy activation to pull the ACT table load (~2.7us) into the DMA
        # wait instead of the first real tanh.
        scratch = smpool.tile([P, 1], F32, tag="scr", name="scratch")
        nc.scalar.activation(scratch[:], warm_sb[:, 0:1], AF.Tanh)

        # --- PE warm-up: 8 small DoubleRow matmuls on the zero tile
        # (~3.4us at the gated 1.2 GHz clock, enough to lift the HAM gate
        # without delaying the first real matmul past chunk-0 arrival). ---
        warm_ps = pspool.tile([P, 2 * FH], F32, tag="ps", name="warm_ps")
        wv = warm_sb[:].rearrange("p (j f) -> p j f", j=2)
        for _ in range(8):
            nc.tensor.matmul(
                warm_ps[:, 0:2 * P], lhsT=wv[:, :, 0:P], rhs=wv,
                start=True, stop=True, perf_mode=DR,
            )

        def mm(pst, t, c):
            mtv = mt_tiles[c][:].rearrange("p (j o) -> p j o", j=2)
            xv = x_tiles[c][:].rearrange("p (j f) -> p j f", j=2)
            for h in range(2):
                nc.tensor.matmul(
                    pst[:, h * FH:(h + 1) * FH],
                    lhsT=mtv[:, :, t * P:(t + 1) * P],
                    rhs=xv[:, :, h * FH:(h + 1) * FH],
                    start=(c == 0), stop=(c == C - 1), perf_mode=DR,
                )

        xm16s = {}

        def drain(t, pst):
            # Scalar (otherwise idle) drains PSUM to bf16 SBUF, freeing the
            # accumulator banks and enabling the DVE's 2x all-bf16 mult.
            xm16 = prpool.tile([P, BC * DS], BF16, tag="xm", name=f"xm{t}")
            nc.scalar.copy(xm16[:], pst[:])
            xm16s[t] = xm16

        def postprocess(t, pst):
            direct = t not in xm16s
            src = pst if direct else xm16s[t][:]
            # prod[o, b, (d,s)] = xm * w_syn (broadcast over b), bf16.
            prod = prpool.tile([P, BC * DS], BF16, tag="prod", name=f"prod{t}")
            nc.vector.tensor_mul(
                prod[:].rearrange("p (b f) -> p b f", b=BC),
                src[:].rearrange("p (b f) -> p b f", b=BC) if direct
                else src.rearrange("p (b f) -> p b f", b=BC),
                wsyn_sb[:, t * DS:(t + 1) * DS].unsqueeze(1)
                .broadcast_to([P, BC, DS]),
            )
            pv = prod[:].rearrange("p (bd s) -> p bd s", s=S)
            dp = smpool.tile([P, BD], F32, tag="dp", name=f"dp{t}")
            if t == OT - 1:
                # Last tile: single-hop all-DVE chain (latency over balance).
                nc.vector.tensor_reduce(dp[:], pv, axis=AX.X, op=OP.add)
                bias_eng = nc.vector
            else:
                # s-reduce split: GpSimd folds s 16->8 (one big add), DVE
                # reduces the remaining 8.
                gr1 = smpool.tile([P, BD * 8], F32, tag="gr1", name=f"gr1{t}")
                nc.gpsimd.tensor_add(
                    gr1[:].rearrange("p (bd s) -> p bd s", s=8),
                    pv[:, :, 0:8], pv[:, :, 8:16],
                )
                nc.vector.tensor_reduce(
                    dp[:], gr1[:].rearrange("p (bd s) -> p bd s", s=8),
                    axis=AX.X, op=OP.add,
                )
                bias_eng = nc.gpsimd
            bias_eng.tensor_add(
                dp[:].rearrange("p (b d) -> p b d", d=D),
                dp[:].rearrange("p (b d) -> p b d", d=D),
                pm_sb[:, t * D:(t + 1) * D].unsqueeze(1)
                .broadcast_to([P, BC, D]),
            )
            dend = smpool.tile([P, BD], F32, tag="dend", name=f"dend{t}")
            nc.scalar.activation(dend[:], dp[:], AF.Tanh)
            # soma: * w_dend, reduce over d, sigmoid(+b_soma) -- small, DVE.
            sp = smpool.tile([P, BD], F32, tag="sp", name=f"sp{t}")
            nc.vector.tensor_mul(
                sp[:].rearrange("p (b d) -> p b d", d=D),
                dend[:].rearrange("p (b d) -> p b d", d=D),
                pm_sb[:, PMW1 + t * D:PMW1 + (t + 1) * D].unsqueeze(1)
                .broadcast_to([P, BC, D]),
            )
            soma = smpool.tile([P, BC], F32, tag="soma", name=f"soma{t}")
            nc.vector.tensor_reduce(
                soma[:], sp[:].rearrange("p (b d) -> p b d", d=D),
                axis=AX.X, op=OP.add,
            )
            nc.scalar.activation(
                out_sb[:, t * BC:(t + 1) * BC], soma[:], AF.Sigmoid,
                bias=pm_sb[:, PMB1 + t:PMB1 + t + 1],
            )

        # Two waves of 4 o-tiles (PSUM fits 4 [128,1024] f32 accumulators).
        # Chunk-0 matmuls for each wave ride the DMA stream; then each tile
        # finishes chunks 1..3.  The scalar drain of tile t is emitted ahead
        # of tile t-1's postprocess so the in-order scalar queue stays one
        # tile ahead (copy(t+1) issues before tanh(t)).  Tiles 6 and 7 skip
        # the drain (direct PSUM mult) to shorten the final-tile chain.
        pst = {}
        for t in range(4):
            pst[t] = pspool.tile([P, 2 * FH], F32, tag="ps", name=f"ps{t}")
            mm(pst[t], t, 0)
        for t in range(4):
            for c in range(1, C):
                mm(pst[t], t, c)
            drain(t, pst[t])
            if t >= 1:
                postprocess(t - 1, pst[t - 1])
        for t in range(4, 8):
            pst[t] = pspool.tile([P, 2 * FH], F32, tag="ps", name=f"ps{t}")
            mm(pst[t], t, 0)
        postprocess(3, pst[3])
        for t in range(4, 8):
            for c in range(1, C):
                mm(pst[t], t, c)
            if t < 6:
                drain(t, pst[t])
            if t >= 5:
                postprocess(t - 1, pst[t - 1])
        postprocess(7, pst[7])

        nc.sync.dma_start(out[:, :], out_sb[:])

    if legalize:
        legalize_waits(nc)
    return nc


def get_nc():
    if "nc" not in _NC_CACHE:
        _NC_CACHE["nc"] = build_nc()
    return _NC_CACHE["nc"]


def _pair_chunks(a):
    """[N, F] -> [C*P, 2*F] with row c*128+p holding planes (j=0, j=1) of
    chunk-pair c: out[c*128+p, j*F+f] = a[256c + 128j + p, f]."""
    Fdim = a.shape[1]
    return np.ascontiguousarray(
        a.reshape(C, 2, P, Fdim).transpose(0, 2, 1, 3).reshape(C * P, 2 * Fdim)
    )


def prepare_in_maps(x, matriz_conexao, w_syn, b_dend, w_dend, b_soma):
    f8 = ml_dtypes.float8_e4m3
    x = np.asarray(x, dtype=np.float32)
    mt_np = _pair_chunks(
        np.ascontiguousarray(np.asarray(matriz_conexao, np.float32).T)
    ).astype(f8)
    wsyn_np = np.ascontiguousarray(
        np.asarray(w_syn, np.float32).reshape(OT, P, DS)
        .transpose(1, 0, 2).reshape(P, OT * DS)
    ).astype(ml_dtypes.bfloat16)
    bd = np.asarray(b_dend, np.float32).reshape(OT, P, D).transpose(1, 0, 2).reshape(P, OT * D)
    wd = np.asarray(w_dend, np.float32).reshape(OT, P, D).transpose(1, 0, 2).reshape(P, OT * D)
    bs = np.asarray(b_soma, np.float32).reshape(OT, P).T
    pm_np = np.ascontiguousarray(np.concatenate([bd, wd, bs], axis=1))
    xt = x.transpose(1, 0, 2, 3).reshape(N, B, DS)
    in_maps = []
    for c in range(NCORES):
        xc_np = _pair_chunks(
            np.ascontiguousarray(
                xt[:, c * BC:(c + 1) * BC, :].reshape(N, BC * DS))
        ).astype(f8)
        in_maps.append({"mt": mt_np, "xc": xc_np, "wsyn": wsyn_np, "pm": pm_np})
    return in_maps


def assemble_output(results):
    outs = []
    for c in range(NCORES):
        oc = np.asarray(results[c]["out"])          # [P, OT*BC] = (oi, (t, b))
        outs.append(oc.reshape(P, OT, BC).transpose(2, 1, 0).reshape(BC, N))
    return np.ascontiguousarray(np.concatenate(outs, axis=0).astype(np.float32))


def kernel(x, matriz_conexao, w_syn, b_dend, w_dend, b_soma):
    from concourse.bass_utils import run_bass_kernel_spmd
    in_maps = prepare_in_maps(x, matriz_conexao, w_syn, b_dend, w_dend, b_soma)
    nc = get_nc()
    res = run_bass_kernel_spmd(nc, in_maps, list(range(NCORES)))
    return assemble_output(res.results)
